# revision 1
# baseline (speedup 1.0000x reference)
"""Trainium2 Bass kernel for the GCM aspect-sentiment model.

Sharding: pure data parallelism — batch (32) split across 8 NeuronCores
(4 items/core); embedding table + all weights replicated.

Per-core plan (all matmuls bf16 with fp32 PSUM accumulation):
  - embedding rows gathered from DRAM via indirect DMA, cast bf16,
    PE-transposed to channel-major [D, B*L] padded layout
  - attention via 2nd-order expansion of tanh(cw+aw) in the small aspect
    term: score = U.ty - (V tx (1-tx^2)).ty^2 with U = V(1-tx^2); the
    l-constant term V.tx cancels in softmax.  This removes the
    [B,L1,L2,E] tanh entirely.
  - conv1/conv2 as 3-tap shifted matmuls; asp_w and asp_b folded into the
    aspect half of conv2 on the host.
  - highway + maxpool + classifier on-chip.
"""

import numpy as np
import ml_dtypes

import concourse.bacc as bacc
import concourse.mybir as mybir
import concourse.tile as tile
from concourse.bass import IndirectOffsetOnAxis
from concourse.masks import make_identity
from concourse.bass_utils import run_bass_kernel_spmd

B, L1, L2 = 32, 512, 16
D, C, NCLS = 300, 256, 3
K = 3
VOCAB = 50000
E = D + C
NCORES = 8
BL = B // NCORES          # batch per core
NL = BL * L1              # 2048 context tokens per core
NM = BL * L2              # 64 aspect tokens per core
LP = L1 + 4               # padded per-b stride (zero col at l=-1 and l=512)
MP = L2 + 2               # padded per-b aspect stride

bf16 = mybir.dt.bfloat16
f32 = mybir.dt.float32
i32 = mybir.dt.int32
AF = mybir.ActivationFunctionType
ALU = mybir.AluOpType
AX = mybir.AxisListType
np_bf16 = ml_dtypes.bfloat16

D_TILES = [(0, 128), (128, 128), (256, 44)]
C_TILES = [(0, 128), (128, 128)]
E_TILES = [(0, 128), (128, 128), (256, 128), (384, 128), (512, 44)]

_NC_CACHE = {}


def build_nc(stage=None):
    nc = bacc.Bacc("TRN2", target_bir_lowering=False, debug=False)

    # ---- DRAM I/O ----
    d_ctx_ids = nc.dram_tensor("ctx_ids", [NL, 1], i32, kind="ExternalInput")
    d_asp_ids = nc.dram_tensor("asp_ids", [NM, 1], i32, kind="ExternalInput")
    d_emb = nc.dram_tensor("wordemb", [VOCAB, D], f32, kind="ExternalInput")
    d_w1t = nc.dram_tensor("w1t", [D, E], bf16, kind="ExternalInput")
    d_w2t = nc.dram_tensor("w2t", [C, E], bf16, kind="ExternalInput")
    d_v2 = nc.dram_tensor("v2", [E, 2], f32, kind="ExternalInput")
    d_w3 = nc.dram_tensor("w3t", [D, K * C], bf16, kind="ExternalInput")
    d_w1c = nc.dram_tensor("w1ct", [D, K * C], bf16, kind="ExternalInput")
    d_w2ctx = nc.dram_tensor("w2ctxt", [D, K * C], bf16, kind="ExternalInput")
    d_w2att = nc.dram_tensor("w2attt", [C, K * C], bf16, kind="ExternalInput")
    d_hwt = nc.dram_tensor("hwt", [C, C], bf16, kind="ExternalInput")
    d_hwgt = nc.dram_tensor("hwgt", [C, C], bf16, kind="ExternalInput")
    d_outwt = nc.dram_tensor("outwt", [C, NCLS], bf16, kind="ExternalInput")
    d_bias = nc.dram_tensor("biases", [C, 5], f32, kind="ExternalInput")
    d_outb = nc.dram_tensor("outb", [BL, NCLS], f32, kind="ExternalInput")
    d_out = nc.dram_tensor("out", [BL, NCLS], f32, kind="ExternalOutput")

    with tile.TileContext(nc) as tc:
        _body(nc, tc, d_ctx_ids, d_asp_ids, d_emb, d_w1t, d_w2t, d_v2, d_w3,
              d_w1c, d_w2ctx, d_w2att, d_hwt, d_hwgt, d_outwt, d_bias, d_outb,
              d_out, stage=stage)
    nc.compile()
    return nc


def _body(nc, tc, d_ctx_ids, d_asp_ids, d_emb, d_w1t, d_w2t, d_v2, d_w3,
          d_w1c, d_w2ctx, d_w2att, d_hwt, d_hwgt, d_outwt, d_bias, d_outb,
          d_out, stage=None):
    import contextlib
    stack = contextlib.ExitStack()
    cst = stack.enter_context(tc.tile_pool(name="cst", bufs=1))
    per = stack.enter_context(tc.tile_pool(name="per", bufs=1))
    wk = stack.enter_context(tc.tile_pool(name="wk", bufs=3))
    ps = stack.enter_context(tc.tile_pool(name="ps", bufs=4, space="PSUM"))
    ps2 = stack.enter_context(tc.tile_pool(name="ps2", bufs=4, space="PSUM"))


    def finish(src):
        osb = wk.tile([BL, NCLS], f32, tag="osb", name="osb")
        nc.vector.tensor_copy(osb[:], src)
        nc.sync.dma_start(d_out.ap(), osb[:])
        stack.close()

    # ---- constants into SBUF ----
    ident = cst.tile([128, 128], bf16, tag="ident", name="ident")
    make_identity(nc, ident[:])

    w1t_sb = []
    for dt, (d0, dsz) in enumerate(D_TILES):
        t = cst.tile([dsz, E], bf16, tag=f"w1t{dt}", name=f"w1t{dt}")
        nc.sync.dma_start(t[:], d_w1t.ap()[d0:d0 + dsz, :])
        w1t_sb.append(t)
    w2t_sb = []
    for ct, (c0, csz) in enumerate(C_TILES):
        t = cst.tile([csz, E], bf16, tag=f"w2t{ct}", name=f"w2t{ct}")
        nc.sync.dma_start(t[:], d_w2t.ap()[c0:c0 + csz, :])
        w2t_sb.append(t)
    v2_sb = []
    for et, (e0, esz) in enumerate(E_TILES):
        t = cst.tile([esz, 2], f32, tag=f"v2{et}", name=f"v2{et}")
        nc.sync.dma_start(t[:], d_v2.ap()[e0:e0 + esz, :])
        v2_sb.append(t)

    def load_conv_w(dram, tiles, name):
        out = []
        for it, (o0, osz) in enumerate(tiles):
            t = cst.tile([osz, K * C], bf16, tag=f"{name}{it}", name=f"{name}{it}")
            nc.sync.dma_start(t[:], dram.ap()[o0:o0 + osz, :])
            out.append(t)
        return out

    w3_sb = load_conv_w(d_w3, D_TILES, "w3")
    w1c_sb = load_conv_w(d_w1c, D_TILES, "w1c")
    w2ctx_sb = load_conv_w(d_w2ctx, D_TILES, "w2ctx")
    w2att_sb = load_conv_w(d_w2att, C_TILES, "w2att")

    hwt_sb, hwgt_sb, outwt_sb, bias_sb = [], [], [], []
    for ct, (c0, csz) in enumerate(C_TILES):
        t = cst.tile([csz, C], bf16, tag=f"hwt{ct}", name=f"hwt{ct}")
        nc.sync.dma_start(t[:], d_hwt.ap()[c0:c0 + csz, :])
        hwt_sb.append(t)
        t = cst.tile([csz, C], bf16, tag=f"hwgt{ct}", name=f"hwgt{ct}")
        nc.sync.dma_start(t[:], d_hwgt.ap()[c0:c0 + csz, :])
        hwgt_sb.append(t)
        t = cst.tile([csz, NCLS], bf16, tag=f"outwt{ct}", name=f"outwt{ct}")
        nc.sync.dma_start(t[:], d_outwt.ap()[c0:c0 + csz, :])
        outwt_sb.append(t)
        t = cst.tile([csz, 5], f32, tag=f"bias{ct}", name=f"bias{ct}")
        nc.sync.dma_start(t[:], d_bias.ap()[c0:c0 + csz, :])
        bias_sb.append(t)
    outb_sb = cst.tile([BL, NCLS], f32, tag="outb", name="outb")
    nc.sync.dma_start(outb_sb[:], d_outb.ap())

    # ---- persistent activations ----
    ctxT = [per.tile([dsz, BL * LP], bf16, tag=f"ctxT{dt}", name=f"ctxT{dt}")
            for dt, (d0, dsz) in enumerate(D_TILES)]
    aspT = [per.tile([dsz, BL * MP], bf16, tag=f"aspT{dt}", name=f"aspT{dt}")
            for dt, (d0, dsz) in enumerate(D_TILES)]
    attT = [per.tile([csz, BL * LP], bf16, tag=f"attT{ct}", name=f"attT{ct}")
            for ct, (c0, csz) in enumerate(C_TILES)]
    for t in ctxT + aspT + attT:
        nc.gpsimd.memset(t[:], 0.0)
    UT = [per.tile([esz, NL], bf16, tag=f"UT{et}", name=f"UT{et}")
          for et, (e0, esz) in enumerate(E_TILES)]
    U2T = [per.tile([esz, NL], bf16, tag=f"U2T{et}", name=f"U2T{et}")
           for et, (e0, esz) in enumerate(E_TILES)]
    tyT = [per.tile([esz, NM], bf16, tag=f"tyT{et}", name=f"tyT{et}")
           for et, (e0, esz) in enumerate(E_TILES)]
    nty2T = [per.tile([esz, NM], bf16, tag=f"nty2T{et}", name=f"nty2T{et}")
             for et, (e0, esz) in enumerate(E_TILES)]
    aT = [per.tile([csz, NM], bf16, tag=f"aT{ct}", name=f"aT{ct}")
          for ct, (c0, csz) in enumerate(C_TILES)]
    a_b = [per.tile([L2, C], bf16, tag=f"a_b{b}", name=f"a_b{b}") for b in range(BL)]
    alphaT = per.tile([L2, NL], bf16, tag="alphaT", name="alphaT")
    mT = [per.tile([csz, NL], bf16, tag=f"mT{ct}", name=f"mT{ct}")
          for ct, (c0, csz) in enumerate(C_TILES)]
    pooled = [per.tile([csz, BL], bf16, tag=f"pooled{ct}", name=f"pooled{ct}")
              for ct, (c0, csz) in enumerate(C_TILES)]

    if stage == 0:
        return finish(ident[0:BL, 0:NCLS])

    # ---- context gather + transpose ----
    for t in range(NL // 128):
        b, lc = t // 4, t % 4
        idx = wk.tile([128, 1], i32, tag="idx", name="idx")
        nc.sync.dma_start(idx[:], d_ctx_ids.ap()[t * 128:(t + 1) * 128, :])
        g = wk.tile([128, D], f32, tag="gath", name="gath")
        nc.gpsimd.indirect_dma_start(
            out=g[:], out_offset=None, in_=d_emb.ap(),
            in_offset=IndirectOffsetOnAxis(ap=idx[:, 0:1], axis=0))
        gb = wk.tile([128, D], bf16, tag="gathb", name="gathb")
        nc.vector.tensor_copy(gb[:], g[:])
        for dt, (d0, dsz) in enumerate(D_TILES):
            tr = ps2.tile([128, 128], bf16, tag="sm", name="tr")
            nc.tensor.transpose(out=tr[:dsz, :], in_=gb[:, d0:d0 + dsz],
                                identity=ident[:])
            col = b * LP + 1 + lc * 128
            nc.vector.tensor_copy(ctxT[dt][:, col:col + 128], tr[:dsz, :])

    if stage == 1:
        return finish(ctxT[0][0:BL, 0:NCLS])

    # ---- aspect branch ----
    idxa = wk.tile([NM, 1], i32, tag="idxa", name="idxa")
    nc.sync.dma_start(idxa[:], d_asp_ids.ap())
    ga = wk.tile([NM, D], f32, tag="gatha", name="gatha")
    nc.gpsimd.indirect_dma_start(
        out=ga[:], out_offset=None, in_=d_emb.ap(),
        in_offset=IndirectOffsetOnAxis(ap=idxa[:, 0:1], axis=0))
    gab = wk.tile([NM, D], bf16, tag="gathab", name="gathab")
    nc.vector.tensor_copy(gab[:], ga[:])
    for dt, (d0, dsz) in enumerate(D_TILES):
        tr = ps2.tile([128, 128], bf16, tag="sm", name="tr")
        nc.tensor.transpose(out=tr[:dsz, :NM], in_=gab[:, d0:d0 + dsz],
                            identity=ident[:NM, :NM])
        for b in range(BL):
            nc.vector.tensor_copy(
                aspT[dt][:, b * MP + 1:b * MP + 1 + L2],
                tr[:dsz, b * L2:(b + 1) * L2])

    # conv3 + relu -> aT  (out view [c, b, m])
    for ct, (c0, csz) in enumerate(C_TILES):
        pa = ps2.tile([128, NM], f32, tag="sm", name="pa")
        pa_v = pa[:csz, :].rearrange("p (b m) -> p b m", m=L2)
        first = True
        for k in range(K):
            for dt, (d0, dsz) in enumerate(D_TILES):
                rhs = aspT[dt][:].rearrange("p (b w) -> p b w", w=MP)[:, :, k:k + L2]
                nc.tensor.matmul(pa_v, w3_sb[dt][:, k * C + c0:k * C + c0 + csz],
                                 rhs, start=first,
                                 stop=(k == K - 1 and dt == len(D_TILES) - 1))
                first = False
        nc.scalar.activation(aT[ct][:], pa[:csz, :], AF.Relu,
                             bias=bias_sb[ct][:, 0:1])
    # a_b: per-batch [m, c] copies via transpose
    for b in range(BL):
        for ct, (c0, csz) in enumerate(C_TILES):
            tr = ps2.tile([128, 128], bf16, tag="sm", name="tr")
            nc.tensor.transpose(out=tr[:L2, :csz],
                                in_=aT[ct][:, b * L2:(b + 1) * L2],
                                identity=ident[:csz, :csz])
            nc.vector.tensor_copy(a_b[b][:, c0:c0 + csz], tr[:L2, :csz])
    # aw -> ty, -ty^2
    for et, (e0, esz) in enumerate(E_TILES):
        paw = ps2.tile([128, NM], f32, tag="sm", name="paw")
        for ct, (c0, csz) in enumerate(C_TILES):
            nc.tensor.matmul(paw[:esz, :], w2t_sb[ct][:, e0:e0 + esz], aT[ct][:],
                             start=(ct == 0), stop=(ct == len(C_TILES) - 1))
        nc.scalar.activation(tyT[et][:], paw[:esz, :], AF.Tanh)
        ty2 = wk.tile([128, NM], bf16, tag="ty2", name="ty2")
        nc.vector.tensor_tensor(ty2[:esz, :], tyT[et][:], tyT[et][:], op=ALU.mult)
        nc.vector.tensor_scalar_mul(nty2T[et][:], ty2[:esz, :], -1.0)

    if stage == 2:
        return finish(tyT[0][0:BL, 0:NCLS])

    # ---- cw -> tx -> U, U2 ----
    for b in range(BL):
        for et, (e0, esz) in enumerate(E_TILES):
            pcw = ps.tile([128, L1], f32, tag="mm", name="mm")
            for dt, (d0, dsz) in enumerate(D_TILES):
                nc.tensor.matmul(pcw[:esz, :], w1t_sb[dt][:, e0:e0 + esz],
                                 ctxT[dt][:, b * LP + 1:b * LP + 1 + L1],
                                 start=(dt == 0), stop=(dt == len(D_TILES) - 1))
            tx = wk.tile([128, L1], bf16, tag="tx", name="tx")
            nc.scalar.activation(tx[:esz, :], pcw[:esz, :], AF.Tanh)
            sq = wk.tile([128, L1], bf16, tag="sq", name="sq")
            nc.vector.tensor_tensor(sq[:esz, :], tx[:esz, :], tx[:esz, :],
                                    op=ALU.mult)
            us = UT[et][:, b * L1:(b + 1) * L1]
            nc.vector.tensor_scalar(us, sq[:esz, :],
                                    v2_sb[et][:, 1:2], v2_sb[et][:, 0:1],
                                    op0=ALU.mult, op1=ALU.add)
            nc.vector.tensor_tensor(U2T[et][:, b * L1:(b + 1) * L1],
                                    tx[:esz, :], us, op=ALU.mult)

    if stage == 3:
        return finish(UT[0][0:BL, 0:NCLS])

    # ---- score -> softmax -> alphaT ----
    for b in range(BL):
        for lc in range(L1 // 128):
            psc = ps2.tile([128, L2], f32, tag="sm", name="sc")
            col = b * L1 + lc * 128
            n_et = len(E_TILES)
            for et, (e0, esz) in enumerate(E_TILES):
                nc.tensor.matmul(psc[:], UT[et][:esz, col:col + 128],
                                 tyT[et][:, b * L2:(b + 1) * L2],
                                 start=(et == 0), stop=False)
            for et, (e0, esz) in enumerate(E_TILES):
                nc.tensor.matmul(psc[:], U2T[et][:esz, col:col + 128],
                                 nty2T[et][:, b * L2:(b + 1) * L2],
                                 start=False, stop=(et == n_et - 1))
            al_u = wk.tile([128, L2], bf16, tag="alu", name="alu")
            rs = wk.tile([128, 1], f32, tag="rs", name="rs")
            nc.scalar.activation(al_u[:], psc[:], AF.Exp, accum_out=rs[:])
            rc = wk.tile([128, 1], f32, tag="rc", name="rc")
            nc.vector.reciprocal(rc[:], rs[:])
            al = wk.tile([128, L2], bf16, tag="al", name="al")
            nc.vector.tensor_scalar_mul(al[:], al_u[:], rc[:, 0:1])
            tr = ps2.tile([128, 128], bf16, tag="sm", name="tr")
            nc.tensor.transpose(out=tr[:L2, :], in_=al[:], identity=ident[:])
            nc.vector.tensor_copy(alphaT[:, col:col + 128], tr[:L2, :])

    if stage == 4:
        return finish(alphaT[0:BL, 0:NCLS])

    # ---- att (normalized) ----
    for b in range(BL):
        for ct, (c0, csz) in enumerate(C_TILES):
            pat = ps.tile([128, L1], f32, tag="mm", name="mm")
            nc.tensor.matmul(pat[:csz, :], a_b[b][:, c0:c0 + csz],
                             alphaT[:, b * L1:(b + 1) * L1],
                             start=True, stop=True)
            nc.vector.tensor_copy(
                attT[ct][:, b * LP + 1:b * LP + 1 + L1], pat[:csz, :])

    # ---- conv1 (tanh) and conv2 (relu, asp folded) -> m ----
    for b in range(BL):
        for ct, (c0, csz) in enumerate(C_TILES):
            ps1 = ps.tile([128, L1], f32, tag="mm", name="mm")
            first = True
            for k in range(K):
                for dt, (d0, dsz) in enumerate(D_TILES):
                    nc.tensor.matmul(
                        ps1[:csz, :], w1c_sb[dt][:, k * C + c0:k * C + c0 + csz],
                        ctxT[dt][:, b * LP + k:b * LP + k + L1],
                        start=first, stop=(k == K - 1 and dt == len(D_TILES) - 1))
                    first = False
            pg = ps.tile([128, L1], f32, tag="mm", name="mm")
            first = True
            for k in range(K):
                for dt, (d0, dsz) in enumerate(D_TILES):
                    nc.tensor.matmul(
                        pg[:csz, :], w2ctx_sb[dt][:, k * C + c0:k * C + c0 + csz],
                        ctxT[dt][:, b * LP + k:b * LP + k + L1],
                        start=first, stop=False)
                    first = False
            for k in range(K):
                for jt, (j0, jsz) in enumerate(C_TILES):
                    nc.tensor.matmul(
                        pg[:csz, :], w2att_sb[jt][:, k * C + c0:k * C + c0 + csz],
                        attT[jt][:, b * LP + k:b * LP + k + L1],
                        start=False, stop=(k == K - 1 and jt == len(C_TILES) - 1))
            s1 = wk.tile([128, L1], bf16, tag="s1", name="s1")
            nc.scalar.activation(s1[:csz, :], ps1[:csz, :], AF.Tanh,
                                 bias=bias_sb[ct][:, 1:2])
            gg = wk.tile([128, L1], bf16, tag="gg", name="gg")
            nc.scalar.activation(gg[:csz, :], pg[:csz, :], AF.Relu,
                                 bias=bias_sb[ct][:, 2:3])
            nc.vector.tensor_tensor(mT[ct][:, b * L1:(b + 1) * L1],
                                    s1[:csz, :], gg[:csz, :], op=ALU.mult)

    if stage == 5:
        return finish(mT[0][0:BL, 0:NCLS])

    # ---- highway + maxpool ----
    for b in range(BL):
        for ct, (c0, csz) in enumerate(C_TILES):
            ph = ps.tile([128, L1], f32, tag="mm", name="mm")
            phg = ps.tile([128, L1], f32, tag="mm", name="mm")
            for jt, (j0, jsz) in enumerate(C_TILES):
                nc.tensor.matmul(ph[:csz, :], hwt_sb[jt][:, c0:c0 + csz],
                                 mT[jt][:, b * L1:(b + 1) * L1],
                                 start=(jt == 0), stop=(jt == len(C_TILES) - 1))
            for jt, (j0, jsz) in enumerate(C_TILES):
                nc.tensor.matmul(phg[:csz, :], hwgt_sb[jt][:, c0:c0 + csz],
                                 mT[jt][:, b * L1:(b + 1) * L1],
                                 start=(jt == 0), stop=(jt == len(C_TILES) - 1))
            hh = wk.tile([128, L1], bf16, tag="hh", name="hh")
            nc.scalar.activation(hh[:csz, :], ph[:csz, :], AF.Relu,
                                 bias=bias_sb[ct][:, 3:4])
            gt = wk.tile([128, L1], bf16, tag="gt", name="gt")
            nc.scalar.activation(gt[:csz, :], phg[:csz, :], AF.Sigmoid,
                                 bias=bias_sb[ct][:, 4:5])
            dd = wk.tile([128, L1], bf16, tag="dd", name="dd")
            nc.vector.tensor_tensor(dd[:csz, :], hh[:csz, :],
                                    mT[ct][:, b * L1:(b + 1) * L1], op=ALU.subtract)
            ee = wk.tile([128, L1], bf16, tag="ee", name="ee")
            nc.vector.tensor_tensor(ee[:csz, :], gt[:csz, :], dd[:csz, :],
                                    op=ALU.mult)
            m2 = wk.tile([128, L1], bf16, tag="m2", name="m2")
            nc.vector.tensor_tensor(m2[:csz, :], ee[:csz, :],
                                    mT[ct][:, b * L1:(b + 1) * L1], op=ALU.add)
            nc.vector.reduce_max(out=pooled[ct][:, b:b + 1], in_=m2[:csz, :],
                                 axis=AX.X)

    # ---- classifier ----
    po = ps2.tile([128, L2], f32, tag="sm", name="po")
    for ct, (c0, csz) in enumerate(C_TILES):
        nc.tensor.matmul(po[:BL, :NCLS], pooled[ct][:], outwt_sb[ct][:],
                         start=(ct == 0), stop=(ct == len(C_TILES) - 1))
    osb = wk.tile([BL, NCLS], f32, tag="osb", name="osb")
    nc.vector.tensor_tensor(osb[:], po[:BL, :NCLS], outb_sb[:], op=ALU.add)
    nc.sync.dma_start(d_out.ap(), osb[:])
    stack.close()


def prep_inputs(context_ids, aspect_ids, wordemb, conv3_w, conv3_b, conv1_w,
                conv1_b, conv2_w, conv2_b, attn_W, attn_V, asp_w, asp_b, hw_w,
                hw_b, hwg_w, hwg_b, out_w, out_b):
    """Host-side prep: weight layout transforms + bf16 casts (weights only)."""
    f = np.float32
    attn_W = np.asarray(attn_W, f)
    w2 = np.asarray(conv2_w, f)
    asp_w = np.asarray(asp_w, f)

    shared = {
        "wordemb": np.asarray(wordemb, f),
        "w1t": np.ascontiguousarray(attn_W[:, :D].T).astype(np_bf16),
        "w2t": np.ascontiguousarray(attn_W[:, D:].T).astype(np_bf16),
        "v2": np.stack([np.asarray(attn_V, f)[0], -np.asarray(attn_V, f)[0]],
                       axis=1).astype(f),
        "w3t": np.asarray(conv3_w, f).transpose(1, 2, 0).reshape(D, K * C)
              .astype(np_bf16),
        "w1ct": np.asarray(conv1_w, f).transpose(1, 2, 0).reshape(D, K * C)
               .astype(np_bf16),
        "w2ctxt": w2[:, :D, :].transpose(1, 2, 0).reshape(D, K * C)
                 .astype(np_bf16),
        "w2attt": np.einsum("aok,oc->ack", w2[:, D:, :], asp_w)
                 .transpose(1, 2, 0).reshape(C, K * C).astype(np_bf16),
        "hwt": np.ascontiguousarray(np.asarray(hw_w, f).T).astype(np_bf16),
        "hwgt": np.ascontiguousarray(np.asarray(hwg_w, f).T).astype(np_bf16),
        "outwt": np.ascontiguousarray(np.asarray(out_w, f).T).astype(np_bf16),
        "biases": np.stack([
            np.asarray(conv3_b, f),
            np.asarray(conv1_b, f),
            np.asarray(conv2_b, f) + np.einsum("aok,o->a", w2[:, D:, :],
                                               np.asarray(asp_b, f)),
            np.asarray(hw_b, f),
            np.asarray(hwg_b, f)], axis=1).astype(f),
        "outb": np.tile(np.asarray(out_b, f).reshape(1, NCLS), (BL, 1)),
    }
    in_maps = []
    for c in range(NCORES):
        m = dict(shared)
        m["ctx_ids"] = np.ascontiguousarray(
            np.asarray(context_ids, np.int32)[c * BL:(c + 1) * BL]
        ).reshape(NL, 1)
        m["asp_ids"] = np.ascontiguousarray(
            np.asarray(aspect_ids, np.int32)[c * BL:(c + 1) * BL]
        ).reshape(NM, 1)
        in_maps.append(m)
    return in_maps


def kernel(**inputs):
    if "nc" not in _NC_CACHE:
        _NC_CACHE["nc"] = build_nc()
    nc = _NC_CACHE["nc"]
    in_maps = prep_inputs(**inputs)
    res = run_bass_kernel_spmd(nc, in_maps, core_ids=list(range(NCORES)))
    return np.concatenate([res.results[c]["out"] for c in range(NCORES)], axis=0)


if __name__ == "__main__":
    rng = np.random.default_rng(0)
    print("building...")
    nc = build_nc()
    print("built ok")



# revision 77
# speedup vs baseline: 599.6205x; 599.6205x over previous
"""Trainium2 Bass kernel for the GCM aspect-sentiment model.

Sharding: pure data parallelism — batch (32) split across 8 NeuronCores
(4 items/core); embedding table + all weights replicated.

Per-core plan (all matmuls bf16 with fp32 PSUM accumulation):
  - embedding table pre-cast to bf16 on host; rows gathered from DRAM via
    indirect DMA, PE-transposed to channel-major [D, B*L] padded layout
  - attention via the exact addition formula tanh(x+y)=(tx+ty)/(1+tx*ty)
    truncated at O(ty^3) in the (tiny) aspect term:
      score ~ A1[m] + sum_e tx^2[l,e]*(-V ty)[m,e] + sum_e ((tx^2-1)tx)[l,e]*(V ty^2)[m,e]
    with A1[m] = sum_e V_e ty[m,e] added via a rank-1 ones matmul.  The
    l-constant term V.tx cancels in softmax.  No [B,L1,L2,E] tanh.
  - conv1/conv2 as 3-tap shifted matmuls; asp_w and asp_b folded into the
    aspect half of conv2 on the host.
  - highway + maxpool + classifier on-chip.
  - all weights ship in two packed bf16 tensors + one f32 tensor (3 DMAs);
    index tensors load first so gathers start immediately.
"""

import numpy as np
import ml_dtypes

import concourse.bacc as bacc
import concourse.mybir as mybir
import concourse.tile as tile
from concourse.bass import IndirectOffsetOnAxis
from concourse.masks import make_identity
from concourse.bass_utils import run_bass_kernel_spmd

B, L1, L2 = 32, 512, 16
D, C, NCLS = 300, 256, 3
K = 3
VOCAB = 50000
E = D + C
NCORES = 8
BL = B // NCORES          # batch per core
NL = BL * L1              # 2048 context tokens per core
NM = BL * L2              # 64 aspect tokens per core
LP = L1 + 4               # padded per-b stride (zero cols at l=-1, l=512,513)
MP = L2 + 2               # padded per-b aspect stride

bf16 = mybir.dt.bfloat16
f32 = mybir.dt.float32
i32 = mybir.dt.int32
AF = mybir.ActivationFunctionType
ALU = mybir.AluOpType
AX = mybir.AxisListType
np_bf16 = ml_dtypes.bfloat16

D_TILES = [(0, 128), (128, 128), (256, 44)]
C_TILES = [(0, 128), (128, 128)]
E_TILES = [(0, 128), (128, 128), (256, 128), (384, 128), (512, 44)]

# packed weight layouts (column offsets)
A_W1T = [0, 556, 1112]           # [dsz, E] x3
A_W2T = [1668, 2224]             # [csz, E] x2
A_W3 = [2780, 3548, 4316]        # [dsz, K*C] x3
CA = 5084
B_W1C = [0, 768, 1536]           # [dsz, K*C] x3
B_W2C = [2304, 3072, 3840]       # [dsz, K*C] x3
B_W2A = [4608, 5376]             # [csz, K*C] x2
B_HW = [6144, 6400]              # [csz, C] x2
B_HWG = [6656, 6912]             # [csz, C] x2
B_OUTW = [7168, 7171]            # [csz, NCLS] x2
CB = 7174
# packf: cols 2*et=+V, 2*et+1=-V (et<5); 10+5*ct+j = biases j of tile ct
# (j: conv3,conv1,conv2,hw,hwg); outb at rows 0:BL cols 20:23
CF = 24

_NC_CACHE = {}


def build_nc(debug_taps=False):
    nc = bacc.Bacc("TRN2", target_bir_lowering=False, debug=False)

    d_ctx_idx = nc.dram_tensor("ctx_idx", [128, NL // 128], i32, kind="ExternalInput")
    d_asp_idx = nc.dram_tensor("asp_idx", [NM, 1], i32, kind="ExternalInput")
    d_emb = nc.dram_tensor("wordemb_bf", [VOCAB, D], bf16, kind="ExternalInput")
    d_packa = nc.dram_tensor("packa", [128, CA], bf16, kind="ExternalInput")
    d_packb = nc.dram_tensor("packb", [128, CB], bf16, kind="ExternalInput")
    d_packf = nc.dram_tensor("packf", [128, CF], f32, kind="ExternalInput")
    d_out = nc.dram_tensor("out", [BL, NCLS], f32, kind="ExternalOutput")
    taps = None
    if debug_taps:
        taps = {
            "t_ctxT0": nc.dram_tensor("t_ctxT0", [128, BL * LP], f32, kind="ExternalOutput"),
            "t_aT0": nc.dram_tensor("t_aT0", [128, NM], f32, kind="ExternalOutput"),
            "t_A2n0": nc.dram_tensor("t_A2n0", [128, NM], f32, kind="ExternalOutput"),
            "t_svec": nc.dram_tensor("t_svec", [1, NM], f32, kind="ExternalOutput"),
            "t_sqT0": nc.dram_tensor("t_sqT0", [128, NL], f32, kind="ExternalOutput"),
            "t_QnT0": nc.dram_tensor("t_QnT0", [128, NL], f32, kind="ExternalOutput"),
            "t_alphaT": nc.dram_tensor("t_alphaT", [L2, NL], f32, kind="ExternalOutput"),
            "t_attT0": nc.dram_tensor("t_attT0", [128, BL * LP], f32, kind="ExternalOutput"),
            "t_mT0": nc.dram_tensor("t_mT0", [128, NL], f32, kind="ExternalOutput"),
        }

    with tile.TileContext(nc) as tc:
        _body(nc, tc, d_ctx_idx, d_asp_idx, d_emb, d_packa, d_packb, d_packf,
              d_out, taps=taps)
    nc.compile()
    return nc


def _body(nc, tc, d_ctx_idx, d_asp_idx, d_emb, d_packa, d_packb, d_packf,
          d_out, taps=None):
    import contextlib
    stack = contextlib.ExitStack()
    cst = stack.enter_context(tc.tile_pool(name="cst", bufs=1))
    per = stack.enter_context(tc.tile_pool(name="per", bufs=1))
    wk = stack.enter_context(tc.tile_pool(name="wk", bufs=3))
    gp = stack.enter_context(tc.tile_pool(name="gp", bufs=8))
    ps = stack.enter_context(tc.tile_pool(name="ps", bufs=4, space="PSUM"))
    ps2 = stack.enter_context(tc.tile_pool(name="ps2", bufs=4, space="PSUM"))

    # ---- tiny index loads first so gathers start immediately ----
    idxa = cst.tile([NM, 1], i32, tag="idxa", name="idxa")
    nc.sync.dma_start(idxa[:], d_asp_idx.ap())
    idx = cst.tile([128, NL // 128], i32, tag="idx", name="idx")
    nc.sync.dma_start(idx[:], d_ctx_idx.ap())

    # packa/packf follow the index tensors on the sync DGE queue; packb (not
    # needed until the conv phase) issues on the Pool DGE queue AFTER the
    # gathers so its 1.8MB transfer queues behind them on the DMA engines.
    packa = cst.tile([128, CA], bf16, tag="packa", name="packa")
    nc.sync.dma_start(packa[:], d_packa.ap())
    packb = cst.tile([128, CB], bf16, tag="packb", name="packb")
    packf = cst.tile([128, CF], f32, tag="packf", name="packf")
    nc.sync.dma_start(packf[:], d_packf.ap())

    ident = cst.tile([128, 128], bf16, tag="ident", name="ident")
    mcol = cst.tile([128, 1], bf16, tag="mcol", name="mcol")
    ones_row = cst.tile([1, 128], bf16, tag="ones_row", name="ones_row")

    def pa(dt, off_list, c0, csz):
        o = off_list[dt]
        return packa[0:D_TILES[dt][1], o + c0:o + c0 + csz]

    def vpos(et):
        return packf[0:E_TILES[et][1], 2 * et:2 * et + 1]

    def vneg(et):
        return packf[0:E_TILES[et][1], 2 * et + 1:2 * et + 2]

    def bias(ct, j):
        return packf[0:C_TILES[ct][1], 10 + 5 * ct + j:11 + 5 * ct + j]

    # ---- persistent activations ----
    ctxT = [per.tile([dsz, BL * LP], bf16, tag=f"ctxT{dt}", name=f"ctxT{dt}")
            for dt, (d0, dsz) in enumerate(D_TILES)]
    aspT = [per.tile([dsz, BL * MP], bf16, tag=f"aspT{dt}", name=f"aspT{dt}")
            for dt, (d0, dsz) in enumerate(D_TILES)]
    attT = [per.tile([csz, BL * LP], bf16, tag=f"attT{ct}", name=f"attT{ct}")
            for ct, (c0, csz) in enumerate(C_TILES)]

    sqT = [per.tile([esz, NL], bf16, tag=f"sqT{et}", name=f"sqT{et}")
           for et, (e0, esz) in enumerate(E_TILES)]
    QnT = [per.tile([esz, NL], bf16, tag=f"QnT{et}", name=f"QnT{et}")
           for et, (e0, esz) in enumerate(E_TILES)]
    A2n = [per.tile([esz, NM], bf16, tag=f"A2n{et}", name=f"A2n{et}")
           for et, (e0, esz) in enumerate(E_TILES)]
    A3p = [per.tile([esz, NM], bf16, tag=f"A3p{et}", name=f"A3p{et}")
           for et, (e0, esz) in enumerate(E_TILES)]
    aT = [per.tile([csz, NM], bf16, tag=f"aT{ct}", name=f"aT{ct}")
          for ct, (c0, csz) in enumerate(C_TILES)]
    a_b = [per.tile([L2, C], bf16, tag=f"a_b{b}", name=f"a_b{b}") for b in range(BL)]
    svec = per.tile([1, NM], bf16, tag="svec", name="svec")
    alphaT = per.tile([L2, NL], bf16, tag="alphaT", name="alphaT")
    mT = [per.tile([csz, NL], bf16, tag=f"mT{ct}", name=f"mT{ct}")
          for ct, (c0, csz) in enumerate(C_TILES)]
    pooled = [per.tile([csz, BL], bf16, tag=f"pooled{ct}", name=f"pooled{ct}")
              for ct, (c0, csz) in enumerate(C_TILES)]

    # ---- aspect gather first (small; unblocks the aspect branch) ----
    ga = wk.tile([NM, D], bf16, tag="gatha", name="gatha")
    nc.gpsimd.indirect_dma_start(
        out=ga[:], out_offset=None, in_=d_emb.ap(),
        in_offset=IndirectOffsetOnAxis(ap=idxa[:, 0:1], axis=0))

    # small Pool-engine constants first (identity gates the transposes,
    # mcol/ones gate A1/score, aspT pads gate conv3)
    make_identity(nc, ident[:])
    nc.gpsimd.memset(mcol[:], -1.0)
    nc.gpsimd.memset(ones_row[:], 1.0)
    for t in aspT:
        v = t[:].rearrange("p (b l) -> p b l", l=MP)
        nc.gpsimd.memset(v[:, :, 0:1], 0.0)
        nc.gpsimd.memset(v[:, :, 17:18], 0.0)

    # ---- context gathers.  HW indirect DMA gathers ONE row per partition
    # (extra output width reads consecutive table rows, NOT more indices),
    # so one 128-token gather per instruction. ----
    def gather(t):
        g = gp.tile([128, D], bf16, tag="gath", name="gath")
        nc.gpsimd.indirect_dma_start(
            out=g[:], out_offset=None, in_=d_emb.ap(),
            in_offset=IndirectOffsetOnAxis(ap=idx[:, t:t + 1], axis=0))
        return g

    gctx = [gather(t) for t in range(4)]

    # conv/highway/classifier weights (1.8MB, needed only from the conv
    # phase).  The dummy 1-element copy from an early gather's tile into
    # packb's tile creates a WAR dependency that forces this DMA's transfer
    # to queue BEHIND the first gathers on the DMA engines — DGE queues
    # otherwise dispatch independent DMAs as soon as they're issued.
    nc.gpsimd.tensor_copy(packb[0:1, 0:1], gctx[1][0:1, 0:1])
    nc.sync.dma_start(packb[:], d_packb.ap())

    gctx += [gather(t) for t in range(4, NL // 128)]

    # remaining pad zeroing (read from the conv phase onward)
    for t in ctxT + attT:
        v = t[:].rearrange("p (b l) -> p b l", l=LP)
        nc.gpsimd.memset(v[:, :, 0:1], 0.0)
        nc.gpsimd.memset(v[:, :, 513:515], 0.0)

    def ctx_transpose(b):
        for lc in range(4):
            g = gctx[b * 4 + lc]
            for dt, (d0, dsz) in enumerate(D_TILES):
                tr = ps2.tile([128, 128], bf16, tag="sm", name="tr")
                nc.tensor.transpose(
                    out=tr[:dsz, :], in_=g[:, d0:d0 + dsz],
                    identity=ident[:])
                col = b * LP + 1 + lc * 128
                if dt == 1:
                    nc.scalar.copy(ctxT[dt][:, col:col + 128], tr[:dsz, :])
                else:
                    nc.vector.tensor_copy(ctxT[dt][:, col:col + 128], tr[:dsz, :])

    # ---- aspect branch ----
    for dt, (d0, dsz) in enumerate(D_TILES):
        tr = ps2.tile([128, 128], bf16, tag="sm", name="tr")
        nc.tensor.transpose(out=tr[:dsz, :NM], in_=ga[:, d0:d0 + dsz],
                            identity=ident[:NM, :NM])
        for b in range(BL):
            nc.vector.tensor_copy(
                aspT[dt][:, b * MP + 1:b * MP + 1 + L2],
                tr[:dsz, b * L2:(b + 1) * L2])

    # conv3 + relu -> aT  (out view [c, b, m])
    for ct, (c0, csz) in enumerate(C_TILES):
        pcv = ps2.tile([128, NM], f32, tag="sm", name="pcv")
        pa_v = pcv[:csz, :].rearrange("p (b m) -> p b m", m=L2)
        first = True
        for k in range(K):
            for dt, (d0, dsz) in enumerate(D_TILES):
                rhs = aspT[dt][:].rearrange("p (b w) -> p b w", w=MP)[:, :, k:k + L2]
                nc.tensor.matmul(pa_v, pa(dt, A_W3, k * C + c0, csz),
                                 rhs, start=first,
                                 stop=(k == K - 1 and dt == len(D_TILES) - 1))
                first = False
        nc.scalar.activation(aT[ct][:], pcv[:csz, :], AF.Relu, bias=bias(ct, 0))
    # a_b: per-batch [m, c] copies via transpose
    for b in range(BL):
        for ct, (c0, csz) in enumerate(C_TILES):
            tr = ps2.tile([128, 128], bf16, tag="sm", name="tr")
            nc.tensor.transpose(out=tr[:L2, :csz],
                                in_=aT[ct][:, b * L2:(b + 1) * L2],
                                identity=ident[:csz, :csz])
            nc.vector.tensor_copy(a_b[b][:, c0:c0 + csz], tr[:L2, :csz])
    # aw -> ty -> A2n=-V*ty, A3p=V*ty^2
    for et, (e0, esz) in enumerate(E_TILES):
        paw = ps2.tile([128, NM], f32, tag="sm", name="paw")
        for ct, (c0, csz) in enumerate(C_TILES):
            nc.tensor.matmul(paw[:esz, :],
                             packa[0:csz, A_W2T[ct] + e0:A_W2T[ct] + e0 + esz],
                             aT[ct][:],
                             start=(ct == 0), stop=(ct == len(C_TILES) - 1))
        ty = wk.tile([128, NM], bf16, tag="ty", name="ty")
        nc.scalar.activation(ty[:esz, :], paw[:esz, :], AF.Tanh)
        nc.vector.tensor_scalar_mul(A2n[et][:], ty[:esz, :], vneg(et))
        ty2 = wk.tile([128, NM], bf16, tag="ty2", name="ty2")
        nc.vector.tensor_tensor(ty2[:esz, :], ty[:esz, :], ty[:esz, :],
                                op=ALU.mult)
        nc.vector.tensor_scalar_mul(A3p[et][:], ty2[:esz, :], vpos(et))
    # A1[m] = sum_e V*ty = sum_e (-1)*A2n  (rank-1 matmuls into [1, NM])
    pA1 = ps2.tile([128, NM], f32, tag="sm", name="pA1")
    for et, (e0, esz) in enumerate(E_TILES):
        nc.tensor.matmul(pA1[0:1, :], mcol[:esz, 0:1], A2n[et][:],
                         start=(et == 0), stop=(et == len(E_TILES) - 1))
    nc.vector.tensor_copy(svec[:], pA1[0:1, :])

    def highway(b, chunks=1):
        # jt-outer accumulation: the first half of the matmuls only needs
        # mT[0], which is ready while conv of tile 1 is still running.
        # chunks=2 halves the post-matmul chain for the last item (tail trim).
        W = L1 // chunks
        pooled_parts = []
        for ch in range(chunks):
            s = b * L1 + ch * W
            pp = [(ps.tile([128, W], f32, tag="mm",
                           name="mm"),
                   ps.tile([128, W], f32, tag="mm",
                           name="mm"))
                  for ct in C_TILES]
            nj = len(C_TILES)
            for jt, (j0, jsz) in enumerate(C_TILES):
                for ct, (c0, csz) in enumerate(C_TILES):
                    nc.tensor.matmul(pp[ct][0][:csz, :],
                                     packb[0:jsz, B_HW[jt] + c0:B_HW[jt] + c0 + csz],
                                     mT[jt][:, s:s + W],
                                     start=(jt == 0), stop=(jt == nj - 1))
                    nc.tensor.matmul(pp[ct][1][:csz, :],
                                     packb[0:jsz, B_HWG[jt] + c0:B_HWG[jt] + c0 + csz],
                                     mT[jt][:, s:s + W],
                                     start=(jt == 0), stop=(jt == nj - 1))
            for ct, (c0, csz) in enumerate(C_TILES):
                ph, phg = pp[ct]
                hh = wk.tile([128, W], bf16, tag="hh", name="hh")
                nc.scalar.activation(hh[:csz, :], ph[:csz, :], AF.Relu,
                                     bias=bias(ct, 3))
                # gate = sigmoid(x) = 0.5*tanh(x/2) + 0.5 — stays in the
                # exp/tanh table set (avoids ~1.3us sigmoid table loads).
                # bias col 4 holds 0.5*hwg_b (host side); scale=0.5 halves x.
                gth = wk.tile([128, W], bf16, tag="gt", name="gt")
                nc.scalar.activation(gth[:csz, :], phg[:csz, :], AF.Tanh,
                                     bias=bias(ct, 4), scale=0.5)
                dd = wk.tile([128, W], bf16, tag="dd", name="dd")
                nc.vector.tensor_tensor(dd[:csz, :], hh[:csz, :],
                                        mT[ct][:, s:s + W],
                                        op=ALU.subtract)
                # ee2 = (tanh+1)*dd;  m2 = 0.5*ee2 + m == m + sigmoid*(h-m)
                ee = wk.tile([128, W], bf16, tag="ee", name="ee")
                nc.vector.scalar_tensor_tensor(ee[:csz, :], gth[:csz, :], 1.0,
                                               dd[:csz, :], op0=ALU.add,
                                               op1=ALU.mult)
                m2 = wk.tile([128, W], bf16, tag="m2", name="m2")
                nc.vector.scalar_tensor_tensor(m2[:csz, :], ee[:csz, :], 0.5,
                                               mT[ct][:, s:s + W],
                                               op0=ALU.mult, op1=ALU.add)
                if chunks == 1:
                    nc.vector.reduce_max(out=pooled[ct][:, b:b + 1],
                                         in_=m2[:csz, :], axis=AX.X)
                else:
                    part = wk.tile([128, 2], f32, tag="pmx", name="pmx")
                    nc.vector.reduce_max(out=part[:csz, ch:ch + 1],
                                         in_=m2[:csz, :], axis=AX.X)
                    pooled_parts.append((ct, part))
                    if ch == chunks - 1:
                        pm = [p for (c, p) in pooled_parts if c == ct]
                        nc.vector.tensor_tensor(pooled[ct][:, b:b + 1],
                                                pm[0][:csz, 0:1],
                                                pm[-1][:csz, 1:2], op=ALU.max)

    # ==== fused per-batch-item pipeline (software-pipelined blocks) ====
    def cw_block(b):
        # cw -> tx -> sq, Qn=(sq-1)*tx
        for et, (e0, esz) in enumerate(E_TILES):
            pcw = ps.tile([128, L1], f32, tag="mm", name="mm")
            for dt, (d0, dsz) in enumerate(D_TILES):
                nc.tensor.matmul(pcw[:esz, :],
                                 pa(dt, A_W1T, e0, esz),
                                 ctxT[dt][:, b * LP + 1:b * LP + 1 + L1],
                                 start=(dt == 0), stop=(dt == len(D_TILES) - 1))
            tx = wk.tile([128, L1], bf16, tag="tx", name="tx")
            nc.scalar.activation(tx[:esz, :], pcw[:esz, :], AF.Tanh)
            sq = sqT[et][:, b * L1:(b + 1) * L1]
            nc.vector.tensor_tensor(sq, tx[:esz, :], tx[:esz, :], op=ALU.mult)
            nc.vector.scalar_tensor_tensor(
                QnT[et][:, b * L1:(b + 1) * L1], sq, 1.0, tx[:esz, :],
                op0=ALU.subtract, op1=ALU.mult)

    def score_block(b):
        als = []
        for lc in range(L1 // 128):
            psc = ps2.tile([128, L2], f32, tag="sm", name="sc")
            col = b * L1 + lc * 128
            nc.tensor.matmul(psc[:], ones_row[0:1, :],
                             svec[0:1, b * L2:(b + 1) * L2],
                             start=True, stop=False)
            for et, (e0, esz) in enumerate(E_TILES):
                nc.tensor.matmul(psc[:], sqT[et][:, col:col + 128],
                                 A2n[et][:, b * L2:(b + 1) * L2],
                                 start=False, stop=False)
            n_et = len(E_TILES)
            for et, (e0, esz) in enumerate(E_TILES):
                nc.tensor.matmul(psc[:], QnT[et][:, col:col + 128],
                                 A3p[et][:, b * L2:(b + 1) * L2],
                                 start=False, stop=(et == n_et - 1))
            al_u = wk.tile([128, L2], bf16, tag="alu", name="alu")
            rs = wk.tile([128, 1], f32, tag="rs", name="rs")
            nc.scalar.activation(al_u[:], psc[:], AF.Exp, accum_out=rs[:])
            rc = wk.tile([128, 1], f32, tag="rc", name="rc")
            nc.vector.reciprocal(rc[:], rs[:])
            al = gp.tile([128, L2], bf16, tag="al", name="al")
            nc.vector.tensor_scalar_mul(al[:], al_u[:], rc[:, 0:1])
            als.append(al)
        return als

    def alpha_tr(b, als):
        for lc in range(L1 // 128):
            col = b * L1 + lc * 128
            tr = ps2.tile([128, 128], bf16, tag="sm", name="tr")
            nc.tensor.transpose(out=tr[:L2, :], in_=als[lc][:], identity=ident[:])
            nc.vector.tensor_copy(alphaT[:, col:col + 128], tr[:L2, :])

    def att_block(b):
        for ct, (c0, csz) in enumerate(C_TILES):
            pat = ps.tile([128, L1], f32, tag="mm", name="mm")
            nc.tensor.matmul(pat[:csz, :], a_b[b][:, c0:c0 + csz],
                             alphaT[:, b * L1:(b + 1) * L1],
                             start=True, stop=True)
            nc.vector.tensor_copy(
                attT[ct][:, b * LP + 1:b * LP + 1 + L1], pat[:csz, :])

    def conv_block(b):
        # conv1 (tanh) and conv2 (relu, asp folded) -> m
        for ct, (c0, csz) in enumerate(C_TILES):
            ps1 = ps.tile([128, L1], f32, tag="mm", name="mm")
            first = True
            for k in range(K):
                for dt, (d0, dsz) in enumerate(D_TILES):
                    nc.tensor.matmul(
                        ps1[:csz, :],
                        packb[0:dsz, B_W1C[dt] + k * C + c0:B_W1C[dt] + k * C + c0 + csz],
                        ctxT[dt][:, b * LP + k:b * LP + k + L1],
                        start=first, stop=(k == K - 1 and dt == len(D_TILES) - 1))
                    first = False
            pg = ps.tile([128, L1], f32, tag="mm", name="mm")
            first = True
            for k in range(K):
                for dt, (d0, dsz) in enumerate(D_TILES):
                    nc.tensor.matmul(
                        pg[:csz, :],
                        packb[0:dsz, B_W2C[dt] + k * C + c0:B_W2C[dt] + k * C + c0 + csz],
                        ctxT[dt][:, b * LP + k:b * LP + k + L1],
                        start=first, stop=False)
                    first = False
            for k in range(K):
                for jt, (j0, jsz) in enumerate(C_TILES):
                    nc.tensor.matmul(
                        pg[:csz, :],
                        packb[0:jsz, B_W2A[jt] + k * C + c0:B_W2A[jt] + k * C + c0 + csz],
                        attT[jt][:, b * LP + k:b * LP + k + L1],
                        start=False, stop=(k == K - 1 and jt == len(C_TILES) - 1))
            s1 = wk.tile([128, L1], bf16, tag="s1", name="s1")
            nc.scalar.activation(s1[:csz, :], ps1[:csz, :], AF.Tanh,
                                 bias=bias(ct, 1))
            gg = wk.tile([128, L1], bf16, tag="gg", name="gg")
            nc.scalar.activation(gg[:csz, :], pg[:csz, :], AF.Relu,
                                 bias=bias(ct, 2))
            nc.vector.tensor_tensor(mT[ct][:, b * L1:(b + 1) * L1],
                                    s1[:csz, :], gg[:csz, :], op=ALU.mult)

    for b in range(BL):
        ctx_transpose(b)
        cw_block(b)
        alpha_tr(b, score_block(b))
        att_block(b)
        conv_block(b)
        highway(b)

    # ---- classifier ----
    po = ps2.tile([128, L2], f32, tag="sm", name="po")
    for ct, (c0, csz) in enumerate(C_TILES):
        nc.tensor.matmul(po[:BL, :NCLS], pooled[ct][:],
                         packb[0:csz, B_OUTW[ct]:B_OUTW[ct] + NCLS],
                         start=(ct == 0), stop=(ct == len(C_TILES) - 1))
    osb = wk.tile([BL, NCLS], f32, tag="osb", name="osb")
    nc.vector.tensor_tensor(osb[:], po[:BL, :NCLS], packf[0:BL, 20:20 + NCLS],
                            op=ALU.add)
    nc.sync.dma_start(d_out.ap(), osb[:])
    if taps is not None:
        def dump(dram, tl, p, cols):
            t = per.tile([p, cols], f32, tag=f"dmp{dram.name}", name=f"dmp{dram.name}")
            nc.vector.tensor_copy(t[:], tl)
            nc.sync.dma_start(dram.ap()[0:p, :], t[:])
        dump(taps["t_ctxT0"], ctxT[0][:], 128, BL * LP)
        dump(taps["t_aT0"], aT[0][:], 128, NM)
        dump(taps["t_A2n0"], A2n[0][:], 128, NM)
        dump(taps["t_svec"], svec[:], 1, NM)
        dump(taps["t_sqT0"], sqT[0][:], 128, NL)
        dump(taps["t_QnT0"], QnT[0][:], 128, NL)
        dump(taps["t_alphaT"], alphaT[:], L2, NL)
        dump(taps["t_attT0"], attT[0][:], 128, BL * LP)
        dump(taps["t_mT0"], mT[0][:], 128, NL)
    stack.close()


def prep_inputs(context_ids, aspect_ids, wordemb, conv3_w, conv3_b, conv1_w,
                conv1_b, conv2_w, conv2_b, attn_W, attn_V, asp_w, asp_b, hw_w,
                hw_b, hwg_w, hwg_b, out_w, out_b):
    """Host-side prep: weight packing + bf16 casts (weights + embedding)."""
    f = np.float32
    attn_W = np.asarray(attn_W, f)
    w2 = np.asarray(conv2_w, f)
    asp_w = np.asarray(asp_w, f)
    V = np.asarray(attn_V, f)[0]  # [E]

    def tile_rows(packed, col, mat, tiles):
        for (o0, osz), off in zip(tiles, col):
            packed[0:osz, off:off + mat.shape[1]] = mat[o0:o0 + osz]

    packa = np.zeros((128, CA), np_bf16)
    tile_rows(packa, A_W1T, np.ascontiguousarray(attn_W[:, :D].T).astype(np_bf16), D_TILES)
    tile_rows(packa, A_W2T, np.ascontiguousarray(attn_W[:, D:].T).astype(np_bf16), C_TILES)
    w3t = np.asarray(conv3_w, f).transpose(1, 2, 0).reshape(D, K * C).astype(np_bf16)
    tile_rows(packa, A_W3, w3t, D_TILES)

    packb = np.zeros((128, CB), np_bf16)
    w1ct = np.asarray(conv1_w, f).transpose(1, 2, 0).reshape(D, K * C).astype(np_bf16)
    tile_rows(packb, B_W1C, w1ct, D_TILES)
    w2ctxt = w2[:, :D, :].transpose(1, 2, 0).reshape(D, K * C).astype(np_bf16)
    tile_rows(packb, B_W2C, w2ctxt, D_TILES)
    w2attt = (np.einsum("aok,oc->ack", w2[:, D:, :], asp_w)
              .transpose(1, 2, 0).reshape(C, K * C).astype(np_bf16))
    tile_rows(packb, B_W2A, w2attt, C_TILES)
    tile_rows(packb, B_HW, np.ascontiguousarray(np.asarray(hw_w, f).T).astype(np_bf16), C_TILES)
    tile_rows(packb, B_HWG, np.ascontiguousarray(np.asarray(hwg_w, f).T).astype(np_bf16), C_TILES)
    tile_rows(packb, B_OUTW, np.ascontiguousarray(np.asarray(out_w, f).T).astype(np_bf16), C_TILES)

    packf = np.zeros((128, CF), f)
    for et, (e0, esz) in enumerate(E_TILES):
        packf[0:esz, 2 * et] = V[e0:e0 + esz]
        packf[0:esz, 2 * et + 1] = -V[e0:e0 + esz]
    biases = np.stack([
        np.asarray(conv3_b, f),
        np.asarray(conv1_b, f),
        np.asarray(conv2_b, f) + np.einsum("aok,o->a", w2[:, D:, :],
                                           np.asarray(asp_b, f)),
        np.asarray(hw_b, f),
        0.5 * np.asarray(hwg_b, f)], axis=1)  # [C, 5]
    for ct, (c0, csz) in enumerate(C_TILES):
        packf[0:csz, 10 + 5 * ct:15 + 5 * ct] = biases[c0:c0 + csz]
    packf[0:BL, 20:20 + NCLS] = np.asarray(out_b, f).reshape(1, NCLS)

    shared = {
        "wordemb_bf": np.asarray(wordemb, f).astype(np_bf16),
        "packa": packa,
        "packb": packb,
        "packf": packf,
    }
    in_maps = []
    for c in range(NCORES):
        m = dict(shared)
        ids = np.asarray(context_ids, np.int32)[c * BL:(c + 1) * BL].reshape(NL)
        m["ctx_idx"] = np.ascontiguousarray(ids.reshape(NL // 128, 128).T)
        m["asp_idx"] = np.ascontiguousarray(
            np.asarray(aspect_ids, np.int32)[c * BL:(c + 1) * BL]
        ).reshape(NM, 1)
        in_maps.append(m)
    return in_maps


def kernel(**inputs):
    if "nc" not in _NC_CACHE:
        _NC_CACHE["nc"] = build_nc()
    nc = _NC_CACHE["nc"]
    in_maps = prep_inputs(**inputs)
    res = run_bass_kernel_spmd(nc, in_maps, core_ids=list(range(NCORES)))
    return np.concatenate([res.results[c]["out"] for c in range(NCORES)], axis=0)


if __name__ == "__main__":
    print("building...")
    nc = build_nc()
    print("built ok")


# revision 81
# speedup vs baseline: 710.2261x; 1.1845x over previous
"""Trainium2 Bass kernel for the GCM aspect-sentiment model.

Sharding: pure data parallelism — batch (32) split across 8 NeuronCores
(4 items/core); embedding table + all weights replicated.

Per-core plan (all matmuls bf16 with fp32 PSUM accumulation):
  - embedding table pre-cast to bf16 on host; rows gathered from DRAM via
    indirect DMA, PE-transposed to channel-major [D, B*L] padded layout
  - attention via the exact addition formula tanh(x+y)=(tx+ty)/(1+tx*ty)
    truncated at O(ty^3) in the (tiny) aspect term:
      score ~ A1[m] + sum_e tx^2[l,e]*(-V ty)[m,e] + sum_e ((tx^2-1)tx)[l,e]*(V ty^2)[m,e]
    with A1[m] = sum_e V_e ty[m,e] added via a rank-1 ones matmul.  The
    l-constant term V.tx cancels in softmax.  No [B,L1,L2,E] tanh.
  - conv1/conv2 as 3-tap shifted matmuls; asp_w and asp_b folded into the
    aspect half of conv2 on the host.
  - highway + maxpool + classifier on-chip.
  - all weights ship in two packed bf16 tensors + one f32 tensor (3 DMAs);
    index tensors load first so gathers start immediately.
"""

import numpy as np
import ml_dtypes

import concourse.bacc as bacc
import concourse.mybir as mybir
import concourse.tile as tile
from concourse.bass import IndirectOffsetOnAxis
from concourse.masks import make_identity
from concourse.bass_utils import run_bass_kernel_spmd

B, L1, L2 = 32, 512, 16
D, C, NCLS = 300, 256, 3
K = 3
VOCAB = 50000
E = D + C
NCORES = 8
BL = B // NCORES          # batch per core
NL = BL * L1              # 2048 context tokens per core
NM = BL * L2              # 64 aspect tokens per core
LP = L1 + 4               # padded per-b stride (zero cols at l=-1, l=512,513)
MP = L2 + 2               # padded per-b aspect stride

bf16 = mybir.dt.bfloat16
f32 = mybir.dt.float32
i32 = mybir.dt.int32
AF = mybir.ActivationFunctionType
ALU = mybir.AluOpType
AX = mybir.AxisListType
np_bf16 = ml_dtypes.bfloat16

D_TILES = [(0, 128), (128, 128), (256, 44)]
C_TILES = [(0, 128), (128, 128)]
E_TILES = [(0, 128), (128, 128), (256, 128), (384, 128), (512, 44)]

# packed weight layouts (column offsets)
A_W1T = [0, 556, 1112]           # [dsz, E] x3
A_W2T = [1668, 2224]             # [csz, E] x2
A_W3 = [2780, 3548, 4316]        # [dsz, K*C] x3
CA = 5084
B_W1C = [0, 768, 1536]           # [dsz, K*C] x3
B_W2C = [2304, 3072, 3840]       # [dsz, K*C] x3
B_W2A = [4608, 5376]             # [csz, K*C] x2
B_HW = [6144, 6400]              # [csz, C] x2
B_HWG = [6656, 6912]             # [csz, C] x2
B_OUTW = [7168, 7171]            # [csz, NCLS] x2
S_W1C = 7174                     # [88, C]: conv1 D-remainder, taps k=0/k=1 stacked
S_W2C = 7430                     # [88, C]: conv2-ctx D-remainder, k=0/k=1 stacked
CB = 7686
# packf: cols 2*et=+V, 2*et+1=-V (et<5); 10+5*ct+j = biases j of tile ct
# (j: conv3,conv1,conv2,hw,hwg); outb at rows 0:BL cols 20:23
CF = 24

_NC_CACHE = {}


def build_nc(debug_taps=False):
    nc = bacc.Bacc("TRN2", target_bir_lowering=False, debug=False)

    d_ctx_idx = nc.dram_tensor("ctx_idx", [128, NL // 128], i32, kind="ExternalInput")
    d_asp_idx = nc.dram_tensor("asp_idx", [NM, 1], i32, kind="ExternalInput")
    d_emb = nc.dram_tensor("wordemb_bf", [VOCAB, D], bf16, kind="ExternalInput")
    d_packa = nc.dram_tensor("packa", [128, CA], bf16, kind="ExternalInput")
    d_packb = nc.dram_tensor("packb", [128, CB], bf16, kind="ExternalInput")
    d_packf = nc.dram_tensor("packf", [128, CF], f32, kind="ExternalInput")
    d_out = nc.dram_tensor("out", [BL, NCLS], f32, kind="ExternalOutput")
    taps = None
    if debug_taps:
        taps = {
            "t_ctxT0": nc.dram_tensor("t_ctxT0", [128, BL * LP], f32, kind="ExternalOutput"),
            "t_aT0": nc.dram_tensor("t_aT0", [128, NM], f32, kind="ExternalOutput"),
            "t_A2n0": nc.dram_tensor("t_A2n0", [128, NM], f32, kind="ExternalOutput"),
            "t_svec": nc.dram_tensor("t_svec", [1, NM], f32, kind="ExternalOutput"),
            "t_sqT0": nc.dram_tensor("t_sqT0", [128, NL], f32, kind="ExternalOutput"),
            "t_QnT0": nc.dram_tensor("t_QnT0", [128, NL], f32, kind="ExternalOutput"),
            "t_alphaT": nc.dram_tensor("t_alphaT", [L2, NL], f32, kind="ExternalOutput"),
            "t_attT0": nc.dram_tensor("t_attT0", [128, BL * LP], f32, kind="ExternalOutput"),
            "t_mT0": nc.dram_tensor("t_mT0", [128, NL], f32, kind="ExternalOutput"),
        }

    with tile.TileContext(nc) as tc:
        _body(nc, tc, d_ctx_idx, d_asp_idx, d_emb, d_packa, d_packb, d_packf,
              d_out, taps=taps)
    nc.compile()
    return nc


def _body(nc, tc, d_ctx_idx, d_asp_idx, d_emb, d_packa, d_packb, d_packf,
          d_out, taps=None):
    import contextlib
    stack = contextlib.ExitStack()
    cst = stack.enter_context(tc.tile_pool(name="cst", bufs=1))
    per = stack.enter_context(tc.tile_pool(name="per", bufs=1))
    wk = stack.enter_context(tc.tile_pool(name="wk", bufs=3))
    gp = stack.enter_context(tc.tile_pool(name="gp", bufs=8))
    ps = stack.enter_context(tc.tile_pool(name="ps", bufs=4, space="PSUM"))
    ps2 = stack.enter_context(tc.tile_pool(name="ps2", bufs=4, space="PSUM"))

    # ---- tiny index loads first so gathers start immediately ----
    idxa = cst.tile([NM, 1], i32, tag="idxa", name="idxa")
    nc.sync.dma_start(idxa[:], d_asp_idx.ap())
    idx = cst.tile([128, NL // 128], i32, tag="idx", name="idx")
    nc.sync.dma_start(idx[:], d_ctx_idx.ap())

    # packa/packf follow the index tensors on the sync DGE queue; packb (not
    # needed until the conv phase) issues on the Pool DGE queue AFTER the
    # gathers so its 1.8MB transfer queues behind them on the DMA engines.
    packa = cst.tile([128, CA], bf16, tag="packa", name="packa")
    nc.sync.dma_start(packa[:], d_packa.ap())
    packb = cst.tile([128, CB], bf16, tag="packb", name="packb")
    packf = cst.tile([128, CF], f32, tag="packf", name="packf")
    nc.sync.dma_start(packf[:], d_packf.ap())

    ident = cst.tile([128, 128], bf16, tag="ident", name="ident")
    mcol = cst.tile([128, 1], bf16, tag="mcol", name="mcol")
    ones_row = cst.tile([1, 128], bf16, tag="ones_row", name="ones_row")

    def pa(dt, off_list, c0, csz):
        o = off_list[dt]
        return packa[0:D_TILES[dt][1], o + c0:o + c0 + csz]

    def vpos(et):
        return packf[0:E_TILES[et][1], 2 * et:2 * et + 1]

    def vneg(et):
        return packf[0:E_TILES[et][1], 2 * et + 1:2 * et + 2]

    def bias(ct, j):
        return packf[0:C_TILES[ct][1], 10 + 5 * ct + j:11 + 5 * ct + j]

    # ---- persistent activations ----
    ctxT = [per.tile([dsz, BL * LP], bf16, tag=f"ctxT{dt}", name=f"ctxT{dt}")
            for dt, (d0, dsz) in enumerate(D_TILES)]
    aspT = [per.tile([dsz, BL * MP], bf16, tag=f"aspT{dt}", name=f"aspT{dt}")
            for dt, (d0, dsz) in enumerate(D_TILES)]
    attT = [per.tile([csz, BL * LP], bf16, tag=f"attT{ct}", name=f"attT{ct}")
            for ct, (c0, csz) in enumerate(C_TILES)]
    # D-remainder rows (256:300) with taps k=0 (rows 0:44) and k=1 (rows
    # 64:108) stacked on partitions (compute-engine partition bases must be
    # 32-aligned; rows 44:64 are zeroed on both operands): one 108-row conv
    # pass replaces two 44-row passes for conv1 and conv2-ctx
    stk = per.tile([108, BL * LP], bf16, tag="stk", name="stk")

    sqT = [per.tile([esz, NL], bf16, tag=f"sqT{et}", name=f"sqT{et}")
           for et, (e0, esz) in enumerate(E_TILES)]
    QnT = [per.tile([esz, NL], bf16, tag=f"QnT{et}", name=f"QnT{et}")
           for et, (e0, esz) in enumerate(E_TILES)]
    A2n = [per.tile([esz, NM], bf16, tag=f"A2n{et}", name=f"A2n{et}")
           for et, (e0, esz) in enumerate(E_TILES)]
    A3p = [per.tile([esz, NM], bf16, tag=f"A3p{et}", name=f"A3p{et}")
           for et, (e0, esz) in enumerate(E_TILES)]
    aT = [per.tile([csz, NM], bf16, tag=f"aT{ct}", name=f"aT{ct}")
          for ct, (c0, csz) in enumerate(C_TILES)]
    a_b = [per.tile([L2, C], bf16, tag=f"a_b{b}", name=f"a_b{b}") for b in range(BL)]
    svec = per.tile([1, NM], bf16, tag="svec", name="svec")
    alphaT = per.tile([L2, NL], bf16, tag="alphaT", name="alphaT")
    mT = [per.tile([csz, NL], bf16, tag=f"mT{ct}", name=f"mT{ct}")
          for ct, (c0, csz) in enumerate(C_TILES)]
    pooled = [per.tile([csz, BL], bf16, tag=f"pooled{ct}", name=f"pooled{ct}")
              for ct, (c0, csz) in enumerate(C_TILES)]

    # ---- aspect gather first (small; unblocks the aspect branch) ----
    ga = wk.tile([NM, D], bf16, tag="gatha", name="gatha")
    nc.gpsimd.indirect_dma_start(
        out=ga[:], out_offset=None, in_=d_emb.ap(),
        in_offset=IndirectOffsetOnAxis(ap=idxa[:, 0:1], axis=0))

    # small Pool-engine constants first (identity gates the transposes,
    # mcol/ones gate A1/score, aspT pads gate conv3)
    make_identity(nc, ident[:])
    nc.gpsimd.memset(mcol[:], -1.0)
    nc.gpsimd.memset(ones_row[:], 1.0)
    for t in aspT:
        v = t[:].rearrange("p (b l) -> p b l", l=MP)
        nc.gpsimd.memset(v[:, :, 0:1], 0.0)
        nc.gpsimd.memset(v[:, :, 17:18], 0.0)

    # ---- context gathers.  HW indirect DMA gathers ONE row per partition
    # (extra output width reads consecutive table rows, NOT more indices),
    # so one 128-token gather per instruction. ----
    def gather(t):
        g = gp.tile([128, D], bf16, tag="gath", name="gath")
        nc.gpsimd.indirect_dma_start(
            out=g[:], out_offset=None, in_=d_emb.ap(),
            in_offset=IndirectOffsetOnAxis(ap=idx[:, t:t + 1], axis=0))
        return g

    gctx = [gather(t) for t in range(4)]

    # conv/highway/classifier weights (1.8MB, needed only from the conv
    # phase).  The dummy 1-element copy from an early gather's tile into
    # packb's tile creates a WAR dependency that forces this DMA's transfer
    # to queue BEHIND the first gathers on the DMA engines — DGE queues
    # otherwise dispatch independent DMAs as soon as they're issued.
    nc.gpsimd.tensor_copy(packb[0:1, 0:1], gctx[1][0:1, 0:1])
    nc.sync.dma_start(packb[:], d_packb.ap())

    gctx += [gather(t) for t in range(4, NL // 128)]

    # remaining pad zeroing (read from the conv phase onward)
    for t in ctxT + attT + [stk]:
        v = t[:].rearrange("p (b l) -> p b l", l=LP)
        nc.gpsimd.memset(v[:, :, 0:1], 0.0)
        nc.gpsimd.memset(v[:, :, 513:515], 0.0)
    nc.gpsimd.memset(stk[32:64, :], 0.0)

    def ctx_transpose(b):
        for lc in range(4):
            g = gctx[b * 4 + lc]
            for dt, (d0, dsz) in enumerate(D_TILES):
                tr = ps2.tile([128, 128], bf16, tag="sm", name="tr")
                nc.tensor.transpose(
                    out=tr[:dsz, :], in_=g[:, d0:d0 + dsz],
                    identity=ident[:])
                col = b * LP + 1 + lc * 128
                if dt == 1:
                    nc.scalar.copy(ctxT[dt][:, col:col + 128], tr[:dsz, :])
                else:
                    nc.vector.tensor_copy(ctxT[dt][:, col:col + 128], tr[:dsz, :])
        # stacked remainder: rows 0:44 = tap k=0, rows 64:108 = tap k=1
        nc.vector.tensor_copy(stk[0:44, b * LP + 1:b * LP + 513],
                              ctxT[2][:, b * LP + 1:b * LP + 513])
        nc.vector.tensor_copy(stk[64:108, b * LP:b * LP + 512],
                              ctxT[2][:, b * LP + 1:b * LP + 513])

    # ---- aspect branch ----
    for dt, (d0, dsz) in enumerate(D_TILES):
        tr = ps2.tile([128, 128], bf16, tag="sm", name="tr")
        nc.tensor.transpose(out=tr[:dsz, :NM], in_=ga[:, d0:d0 + dsz],
                            identity=ident[:NM, :NM])
        for b in range(BL):
            nc.vector.tensor_copy(
                aspT[dt][:, b * MP + 1:b * MP + 1 + L2],
                tr[:dsz, b * L2:(b + 1) * L2])

    # conv3 + relu -> aT  (out view [c, b, m])
    for ct, (c0, csz) in enumerate(C_TILES):
        pcv = ps2.tile([128, NM], f32, tag="sm", name="pcv")
        pa_v = pcv[:csz, :].rearrange("p (b m) -> p b m", m=L2)
        first = True
        for k in range(K):
            for dt, (d0, dsz) in enumerate(D_TILES):
                rhs = aspT[dt][:].rearrange("p (b w) -> p b w", w=MP)[:, :, k:k + L2]
                nc.tensor.matmul(pa_v, pa(dt, A_W3, k * C + c0, csz),
                                 rhs, start=first,
                                 stop=(k == K - 1 and dt == len(D_TILES) - 1))
                first = False
        nc.scalar.activation(aT[ct][:], pcv[:csz, :], AF.Relu, bias=bias(ct, 0))
    # a_b: per-batch [m, c] copies via transpose
    for b in range(BL):
        for ct, (c0, csz) in enumerate(C_TILES):
            tr = ps2.tile([128, 128], bf16, tag="sm", name="tr")
            nc.tensor.transpose(out=tr[:L2, :csz],
                                in_=aT[ct][:, b * L2:(b + 1) * L2],
                                identity=ident[:csz, :csz])
            nc.vector.tensor_copy(a_b[b][:, c0:c0 + csz], tr[:L2, :csz])
    # aw -> ty -> A2n=-V*ty, A3p=V*ty^2
    for et, (e0, esz) in enumerate(E_TILES):
        paw = ps2.tile([128, NM], f32, tag="sm", name="paw")
        for ct, (c0, csz) in enumerate(C_TILES):
            nc.tensor.matmul(paw[:esz, :],
                             packa[0:csz, A_W2T[ct] + e0:A_W2T[ct] + e0 + esz],
                             aT[ct][:],
                             start=(ct == 0), stop=(ct == len(C_TILES) - 1))
        ty = wk.tile([128, NM], bf16, tag="ty", name="ty")
        nc.scalar.activation(ty[:esz, :], paw[:esz, :], AF.Tanh)
        nc.vector.tensor_scalar_mul(A2n[et][:], ty[:esz, :], vneg(et))
        ty2 = wk.tile([128, NM], bf16, tag="ty2", name="ty2")
        nc.vector.tensor_tensor(ty2[:esz, :], ty[:esz, :], ty[:esz, :],
                                op=ALU.mult)
        nc.vector.tensor_scalar_mul(A3p[et][:], ty2[:esz, :], vpos(et))
    # A1[m] = sum_e V*ty = sum_e (-1)*A2n  (rank-1 matmuls into [1, NM])
    pA1 = ps2.tile([128, NM], f32, tag="sm", name="pA1")
    for et, (e0, esz) in enumerate(E_TILES):
        nc.tensor.matmul(pA1[0:1, :], mcol[:esz, 0:1], A2n[et][:],
                         start=(et == 0), stop=(et == len(E_TILES) - 1))
    nc.vector.tensor_copy(svec[:], pA1[0:1, :])

    def highway(b, chunks=1):
        # jt-outer accumulation: the first half of the matmuls only needs
        # mT[0], which is ready while conv of tile 1 is still running.
        # chunks=2 halves the post-matmul chain for the last item (tail trim).
        W = L1 // chunks
        pooled_parts = []
        for ch in range(chunks):
            s = b * L1 + ch * W
            pp = [(ps.tile([128, W], f32, tag="mm",
                           name="mm"),
                   ps.tile([128, W], f32, tag="mm",
                           name="mm"))
                  for ct in C_TILES]
            nj = len(C_TILES)
            for jt, (j0, jsz) in enumerate(C_TILES):
                for ct, (c0, csz) in enumerate(C_TILES):
                    nc.tensor.matmul(pp[ct][0][:csz, :],
                                     packb[0:jsz, B_HW[jt] + c0:B_HW[jt] + c0 + csz],
                                     mT[jt][:, s:s + W],
                                     start=(jt == 0), stop=(jt == nj - 1))
                    nc.tensor.matmul(pp[ct][1][:csz, :],
                                     packb[0:jsz, B_HWG[jt] + c0:B_HWG[jt] + c0 + csz],
                                     mT[jt][:, s:s + W],
                                     start=(jt == 0), stop=(jt == nj - 1))
            for ct, (c0, csz) in enumerate(C_TILES):
                ph, phg = pp[ct]
                hh = wk.tile([128, W], bf16, tag="hh", name="hh")
                nc.scalar.activation(hh[:csz, :], ph[:csz, :], AF.Relu,
                                     bias=bias(ct, 3))
                # gate = sigmoid(x) = 0.5*tanh(x/2) + 0.5 — stays in the
                # exp/tanh table set (avoids ~1.3us sigmoid table loads).
                # bias col 4 holds 0.5*hwg_b (host side); scale=0.5 halves x.
                gth = wk.tile([128, W], bf16, tag="gt", name="gt")
                nc.scalar.activation(gth[:csz, :], phg[:csz, :], AF.Tanh,
                                     bias=bias(ct, 4), scale=0.5)
                dd = wk.tile([128, W], bf16, tag="dd", name="dd")
                nc.vector.tensor_tensor(dd[:csz, :], hh[:csz, :],
                                        mT[ct][:, s:s + W],
                                        op=ALU.subtract)
                # ee2 = (tanh+1)*dd;  m2 = 0.5*ee2 + m == m + sigmoid*(h-m)
                ee = wk.tile([128, W], bf16, tag="ee", name="ee")
                nc.vector.scalar_tensor_tensor(ee[:csz, :], gth[:csz, :], 1.0,
                                               dd[:csz, :], op0=ALU.add,
                                               op1=ALU.mult)
                m2 = wk.tile([128, W], bf16, tag="m2", name="m2")
                nc.vector.scalar_tensor_tensor(m2[:csz, :], ee[:csz, :], 0.5,
                                               mT[ct][:, s:s + W],
                                               op0=ALU.mult, op1=ALU.add)
                if chunks == 1:
                    nc.vector.reduce_max(out=pooled[ct][:, b:b + 1],
                                         in_=m2[:csz, :], axis=AX.X)
                else:
                    part = wk.tile([128, 2], f32, tag="pmx", name="pmx")
                    nc.vector.reduce_max(out=part[:csz, ch:ch + 1],
                                         in_=m2[:csz, :], axis=AX.X)
                    pooled_parts.append((ct, part))
                    if ch == chunks - 1:
                        pm = [p for (c, p) in pooled_parts if c == ct]
                        nc.vector.tensor_tensor(pooled[ct][:, b:b + 1],
                                                pm[0][:csz, 0:1],
                                                pm[-1][:csz, 1:2], op=ALU.max)

    # ==== fused per-batch-item pipeline (software-pipelined blocks) ====
    def cw_block(b):
        # cw -> tx -> sq, Qn=(sq-1)*tx
        for et, (e0, esz) in enumerate(E_TILES):
            pcw = ps.tile([128, L1], f32, tag="mm", name="mm")
            for dt, (d0, dsz) in enumerate(D_TILES):
                nc.tensor.matmul(pcw[:esz, :],
                                 pa(dt, A_W1T, e0, esz),
                                 ctxT[dt][:, b * LP + 1:b * LP + 1 + L1],
                                 start=(dt == 0), stop=(dt == len(D_TILES) - 1))
            tx = wk.tile([128, L1], bf16, tag="tx", name="tx")
            nc.scalar.activation(tx[:esz, :], pcw[:esz, :], AF.Tanh)
            sq = sqT[et][:, b * L1:(b + 1) * L1]
            nc.vector.tensor_tensor(sq, tx[:esz, :], tx[:esz, :], op=ALU.mult)
            nc.vector.scalar_tensor_tensor(
                QnT[et][:, b * L1:(b + 1) * L1], sq, 1.0, tx[:esz, :],
                op0=ALU.subtract, op1=ALU.mult)

    def score_block(b):
        als = []
        for lc in range(L1 // 128):
            psc = ps2.tile([128, L2], f32, tag="sm", name="sc")
            col = b * L1 + lc * 128
            nc.tensor.matmul(psc[:], ones_row[0:1, :],
                             svec[0:1, b * L2:(b + 1) * L2],
                             start=True, stop=False)
            for et, (e0, esz) in enumerate(E_TILES):
                nc.tensor.matmul(psc[:], sqT[et][:, col:col + 128],
                                 A2n[et][:, b * L2:(b + 1) * L2],
                                 start=False, stop=False)
            n_et = len(E_TILES)
            for et, (e0, esz) in enumerate(E_TILES):
                nc.tensor.matmul(psc[:], QnT[et][:, col:col + 128],
                                 A3p[et][:, b * L2:(b + 1) * L2],
                                 start=False, stop=(et == n_et - 1))
            al_u = wk.tile([128, L2], bf16, tag="alu", name="alu")
            rs = wk.tile([128, 1], f32, tag="rs", name="rs")
            nc.scalar.activation(al_u[:], psc[:], AF.Exp, accum_out=rs[:])
            rc = wk.tile([128, 1], f32, tag="rc", name="rc")
            nc.vector.reciprocal(rc[:], rs[:])
            al = gp.tile([128, L2], bf16, tag="al", name="al")
            nc.vector.tensor_scalar_mul(al[:], al_u[:], rc[:, 0:1])
            als.append(al)
        return als

    def alpha_tr(b, als):
        for lc in range(L1 // 128):
            col = b * L1 + lc * 128
            tr = ps2.tile([128, 128], bf16, tag="sm", name="tr")
            nc.tensor.transpose(out=tr[:L2, :], in_=als[lc][:], identity=ident[:])
            nc.vector.tensor_copy(alphaT[:, col:col + 128], tr[:L2, :])

    def att_block(b):
        for ct, (c0, csz) in enumerate(C_TILES):
            pat = ps.tile([128, L1], f32, tag="mm", name="mm")
            nc.tensor.matmul(pat[:csz, :], a_b[b][:, c0:c0 + csz],
                             alphaT[:, b * L1:(b + 1) * L1],
                             start=True, stop=True)
            nc.vector.tensor_copy(
                attT[ct][:, b * LP + 1:b * LP + 1 + L1], pat[:csz, :])

    def conv_block(b):
        # conv1 (tanh) and conv2 (relu, asp folded) -> m
        for ct, (c0, csz) in enumerate(C_TILES):
            ps1 = ps.tile([128, L1], f32, tag="mm", name="mm")
            first = True
            for k in range(K):
                for dt in (0, 1):
                    nc.tensor.matmul(
                        ps1[:csz, :],
                        packb[0:128, B_W1C[dt] + k * C + c0:B_W1C[dt] + k * C + c0 + csz],
                        ctxT[dt][:, b * LP + k:b * LP + k + L1],
                        start=first, stop=False)
                    first = False
            # D-remainder: stacked k0+k1 pass, then the k2 pass
            nc.tensor.matmul(ps1[:csz, :],
                             packb[0:108, S_W1C + c0:S_W1C + c0 + csz],
                             stk[:, b * LP:b * LP + L1],
                             start=False, stop=False)
            nc.tensor.matmul(ps1[:csz, :],
                             packb[0:44, B_W1C[2] + 2 * C + c0:B_W1C[2] + 2 * C + c0 + csz],
                             ctxT[2][:, b * LP + 2:b * LP + 2 + L1],
                             start=False, stop=True)
            pg = ps.tile([128, L1], f32, tag="mm", name="mm")
            first = True
            for k in range(K):
                for dt in (0, 1):
                    nc.tensor.matmul(
                        pg[:csz, :],
                        packb[0:128, B_W2C[dt] + k * C + c0:B_W2C[dt] + k * C + c0 + csz],
                        ctxT[dt][:, b * LP + k:b * LP + k + L1],
                        start=first, stop=False)
                    first = False
            nc.tensor.matmul(pg[:csz, :],
                             packb[0:108, S_W2C + c0:S_W2C + c0 + csz],
                             stk[:, b * LP:b * LP + L1],
                             start=False, stop=False)
            nc.tensor.matmul(pg[:csz, :],
                             packb[0:44, B_W2C[2] + 2 * C + c0:B_W2C[2] + 2 * C + c0 + csz],
                             ctxT[2][:, b * LP + 2:b * LP + 2 + L1],
                             start=False, stop=False)
            for k in range(K):
                for jt, (j0, jsz) in enumerate(C_TILES):
                    nc.tensor.matmul(
                        pg[:csz, :],
                        packb[0:jsz, B_W2A[jt] + k * C + c0:B_W2A[jt] + k * C + c0 + csz],
                        attT[jt][:, b * LP + k:b * LP + k + L1],
                        start=False, stop=(k == K - 1 and jt == len(C_TILES) - 1))
            s1 = wk.tile([128, L1], bf16, tag="s1", name="s1")
            nc.scalar.activation(s1[:csz, :], ps1[:csz, :], AF.Tanh,
                                 bias=bias(ct, 1))
            gg = wk.tile([128, L1], bf16, tag="gg", name="gg")
            nc.scalar.activation(gg[:csz, :], pg[:csz, :], AF.Relu,
                                 bias=bias(ct, 2))
            nc.vector.tensor_tensor(mT[ct][:, b * L1:(b + 1) * L1],
                                    s1[:csz, :], gg[:csz, :], op=ALU.mult)

    for b in range(BL):
        ctx_transpose(b)
        cw_block(b)
        alpha_tr(b, score_block(b))
        att_block(b)
        conv_block(b)
        highway(b)

    # ---- classifier ----
    po = ps2.tile([128, L2], f32, tag="sm", name="po")
    for ct, (c0, csz) in enumerate(C_TILES):
        nc.tensor.matmul(po[:BL, :NCLS], pooled[ct][:],
                         packb[0:csz, B_OUTW[ct]:B_OUTW[ct] + NCLS],
                         start=(ct == 0), stop=(ct == len(C_TILES) - 1))
    osb = wk.tile([BL, NCLS], f32, tag="osb", name="osb")
    nc.vector.tensor_tensor(osb[:], po[:BL, :NCLS], packf[0:BL, 20:20 + NCLS],
                            op=ALU.add)
    nc.sync.dma_start(d_out.ap(), osb[:])
    if taps is not None:
        def dump(dram, tl, p, cols):
            t = per.tile([p, cols], f32, tag=f"dmp{dram.name}", name=f"dmp{dram.name}")
            nc.vector.tensor_copy(t[:], tl)
            nc.sync.dma_start(dram.ap()[0:p, :], t[:])
        dump(taps["t_ctxT0"], ctxT[0][:], 128, BL * LP)
        dump(taps["t_aT0"], aT[0][:], 128, NM)
        dump(taps["t_A2n0"], A2n[0][:], 128, NM)
        dump(taps["t_svec"], svec[:], 1, NM)
        dump(taps["t_sqT0"], sqT[0][:], 128, NL)
        dump(taps["t_QnT0"], QnT[0][:], 128, NL)
        dump(taps["t_alphaT"], alphaT[:], L2, NL)
        dump(taps["t_attT0"], attT[0][:], 128, BL * LP)
        dump(taps["t_mT0"], mT[0][:], 128, NL)
    stack.close()


def prep_inputs(context_ids, aspect_ids, wordemb, conv3_w, conv3_b, conv1_w,
                conv1_b, conv2_w, conv2_b, attn_W, attn_V, asp_w, asp_b, hw_w,
                hw_b, hwg_w, hwg_b, out_w, out_b):
    """Host-side prep: weight packing + bf16 casts (weights + embedding)."""
    f = np.float32
    attn_W = np.asarray(attn_W, f)
    w2 = np.asarray(conv2_w, f)
    asp_w = np.asarray(asp_w, f)
    V = np.asarray(attn_V, f)[0]  # [E]

    def tile_rows(packed, col, mat, tiles):
        for (o0, osz), off in zip(tiles, col):
            packed[0:osz, off:off + mat.shape[1]] = mat[o0:o0 + osz]

    packa = np.zeros((128, CA), np_bf16)
    tile_rows(packa, A_W1T, np.ascontiguousarray(attn_W[:, :D].T).astype(np_bf16), D_TILES)
    tile_rows(packa, A_W2T, np.ascontiguousarray(attn_W[:, D:].T).astype(np_bf16), C_TILES)
    w3t = np.asarray(conv3_w, f).transpose(1, 2, 0).reshape(D, K * C).astype(np_bf16)
    tile_rows(packa, A_W3, w3t, D_TILES)

    packb = np.zeros((128, CB), np_bf16)
    w1ct = np.asarray(conv1_w, f).transpose(1, 2, 0).reshape(D, K * C).astype(np_bf16)
    tile_rows(packb, B_W1C, w1ct, D_TILES)
    w2ctxt = w2[:, :D, :].transpose(1, 2, 0).reshape(D, K * C).astype(np_bf16)
    tile_rows(packb, B_W2C, w2ctxt, D_TILES)
    w2attt = (np.einsum("aok,oc->ack", w2[:, D:, :], asp_w)
              .transpose(1, 2, 0).reshape(C, K * C).astype(np_bf16))
    tile_rows(packb, B_W2A, w2attt, C_TILES)
    packb[0:44, S_W1C:S_W1C + C] = w1ct[256:300, 0:C]
    packb[64:108, S_W1C:S_W1C + C] = w1ct[256:300, C:2 * C]
    packb[0:44, S_W2C:S_W2C + C] = w2ctxt[256:300, 0:C]
    packb[64:108, S_W2C:S_W2C + C] = w2ctxt[256:300, C:2 * C]
    tile_rows(packb, B_HW, np.ascontiguousarray(np.asarray(hw_w, f).T).astype(np_bf16), C_TILES)
    tile_rows(packb, B_HWG, np.ascontiguousarray(np.asarray(hwg_w, f).T).astype(np_bf16), C_TILES)
    tile_rows(packb, B_OUTW, np.ascontiguousarray(np.asarray(out_w, f).T).astype(np_bf16), C_TILES)

    packf = np.zeros((128, CF), f)
    for et, (e0, esz) in enumerate(E_TILES):
        packf[0:esz, 2 * et] = V[e0:e0 + esz]
        packf[0:esz, 2 * et + 1] = -V[e0:e0 + esz]
    biases = np.stack([
        np.asarray(conv3_b, f),
        np.asarray(conv1_b, f),
        np.asarray(conv2_b, f) + np.einsum("aok,o->a", w2[:, D:, :],
                                           np.asarray(asp_b, f)),
        np.asarray(hw_b, f),
        0.5 * np.asarray(hwg_b, f)], axis=1)  # [C, 5]
    for ct, (c0, csz) in enumerate(C_TILES):
        packf[0:csz, 10 + 5 * ct:15 + 5 * ct] = biases[c0:c0 + csz]
    packf[0:BL, 20:20 + NCLS] = np.asarray(out_b, f).reshape(1, NCLS)

    shared = {
        "wordemb_bf": np.asarray(wordemb, f).astype(np_bf16),
        "packa": packa,
        "packb": packb,
        "packf": packf,
    }
    in_maps = []
    for c in range(NCORES):
        m = dict(shared)
        ids = np.asarray(context_ids, np.int32)[c * BL:(c + 1) * BL].reshape(NL)
        m["ctx_idx"] = np.ascontiguousarray(ids.reshape(NL // 128, 128).T)
        m["asp_idx"] = np.ascontiguousarray(
            np.asarray(aspect_ids, np.int32)[c * BL:(c + 1) * BL]
        ).reshape(NM, 1)
        in_maps.append(m)
    return in_maps


def kernel(**inputs):
    if "nc" not in _NC_CACHE:
        _NC_CACHE["nc"] = build_nc()
    nc = _NC_CACHE["nc"]
    in_maps = prep_inputs(**inputs)
    res = run_bass_kernel_spmd(nc, in_maps, core_ids=list(range(NCORES)))
    return np.concatenate([res.results[c]["out"] for c in range(NCORES)], axis=0)


if __name__ == "__main__":
    print("building...")
    nc = build_nc()
    print("built ok")
